# revision 7
# baseline (speedup 1.0000x reference)
"""MoE FFN (top-1 routing) Trainium2 kernel — expert-parallel across 8 cores.

Strategy (per the expert-parallel sharding hint): the router gate and the
token dispatch ARE the sharding step, performed on the host inside kernel():
  - host computes router logits (x @ Wg + bg) and argmax expert ids
  - tokens are gathered per expert, padded to capacity C = max expert load
  - core e receives expert e's W1/W2/b1 plus its routed tokens, pre-tiled
    into contiguous-DMA layouts
  - the device runs the full FFN (both matmuls + exact gelu) in float32r
  - host scatters per-expert outputs back (adds b2 there, it is per-token
    constant) and un-shards to the full [B, S, D] output

Device kernel per core (C tokens, D=1024, H=4096):
  phase A: hT[m*128+j, c] = gelu(sum_k W1tile[k,m].T x^T[k] + b1), m in 0..31
  phase B: yT[d*128+j, c] = sum_k W2tile[k,d].T hT[k],             d in 0..7
Both matmuls consume the weights in their natural [K, M] orientation as the
stationary operand, so no transposes are needed anywhere on the device.

DMA ring assignment: w1 stream on the sync HWDGE ring; xt + w2 stream on the
scalar HWDGE ring; output writes on gpsimd SWDGE.  A short burst of scratch
matmuls at kernel start keeps the PE HAM clock-gate warm through the input
load prologue.
"""

import os
import sys

import numpy as np

for _p in ("/opt/trn_rl_repo", "/root/.axon_site/_ro/trn_rl_repo"):
    if os.path.isdir(_p) and _p not in sys.path:
        sys.path.insert(0, _p)

D_MODEL = 1024
D_HIDDEN = 4096
N_EXPERTS = 8
N_CORES = 8
P = 128
KD = D_MODEL // P  # 8 k-chunks over d_model
MH = D_HIDDEN // P  # 32 m-chunks over d_hidden
MG = 2  # m-chunks per w1 DMA (1 MiB transfers)
N_WARM = 5  # scratch f32 matmuls (~1.7us each cold) to warm the PE clock gate

_compiled_cache = {}

# Set by the most recent kernel() call when BASS_KERNEL_TRACE=1: HW exec ns.
last_exec_time_ns = None
last_results = None


def _chunk_sizes(C):
    """Split C token columns into chunks <= 512, as evenly as possible.

    C >= 512 always (max expert load >= 4096/8), so chunks land in
    [256, 512] and float32r matmuls run at full 1 cycle/row speed.
    """
    nch = -(-C // 512)
    base, rem = divmod(C, nch)
    return [base + 1] * rem + [base] * (nch - rem)


def _build_program(C):
    import concourse.mybir as mybir
    import concourse.tile as tile
    from concourse import bacc

    f32 = mybir.dt.float32
    f32r = mybir.dt.float32r

    nc = bacc.Bacc("TRN2", target_bir_lowering=False, debug=False,
                   num_devices=N_CORES)

    # Host-pretiled inputs (layouts chosen so each DMA is contiguous):
    #   xt  [128, KD*C]            xt[p, k*C + c] = x[c, k*128+p]
    #   w1  [MH/MG, 128, MG*KD*128] w1[g, p, (i*KD+k)*128+j] = W1[k*128+p, (g*MG+i)*128+j]
    #   w2  [KD, 128, MH*128]       w2[d, p, k*128+j] = W2[k*128+p, d*128+j]
    #   b1t [128, MH]               b1t[p, m] = b1[m*128+p]
    # Output:
    #   yt  [KD, 128, C]            yt[d, p, c] = y[c, d*128+p]  (pre-b2)
    xt_d = nc.declare_dram_parameter("xt", [P, KD * C], f32r, isOutput=False)
    w1_d = nc.declare_dram_parameter(
        "w1", [MH // MG, P, MG * KD * P], f32r, isOutput=False)
    w2_d = nc.declare_dram_parameter("w2", [KD, P, MH * P], f32r, isOutput=False)
    b1_d = nc.declare_dram_parameter("b1t", [P, MH], f32, isOutput=False)
    yt_d = nc.declare_dram_parameter("yt", [KD, P, C], f32, isOutput=True)

    chunks = _chunk_sizes(C)

    with tile.TileContext(nc) as tc:
        with (
            tc.tile_pool(name="persist", bufs=1) as persist,
            tc.tile_pool(name="w1p", bufs=3) as w1p,
            tc.tile_pool(name="w2p", bufs=3) as w2p,
            tc.tile_pool(name="outp", bufs=4) as outp,
            tc.tile_pool(name="psum", bufs=6, space="PSUM") as psum,
        ):
            # --- PE warm-up: keep HAM at 8/8 through the input-load prologue
            scratch = persist.tile([P, 512], f32)
            nc.gpsimd.memset(scratch[:], 0.0)
            warm_ps = psum.tile([P, 512], mybir.dt.float32, tag="warm", bufs=1)
            for _ in range(N_WARM):
                nc.tensor.matmul(warm_ps[:], scratch[:, :P], scratch[:],
                                 start=True, stop=True)

            # --- input loads
            xt = persist.tile([P, KD * C], f32r)
            nc.scalar.dma_start(out=xt[:], in_=xt_d[:])
            b1t = persist.tile([P, MH], f32)
            nc.gpsimd.dma_start(out=b1t[:], in_=b1_d[:])
            ht = persist.tile([P, MH * C], f32r)

            # ---- Phase A: hT = gelu(W1^T x^T + b1) ----
            for g in range(MH // MG):
                w1g = w1p.tile([P, MG * KD * P], f32r, tag="w1g")
                nc.sync.dma_start(out=w1g[:], in_=w1_d[g])
                for i in range(MG):
                    m = g * MG + i
                    c0 = 0
                    for cn in chunks:
                        ps = psum.tile([P, 512], mybir.dt.float32, tag="ps")
                        for k in range(KD):
                            nc.tensor.matmul(
                                ps[:, :cn],
                                w1g[:, (i * KD + k) * P:(i * KD + k + 1) * P],
                                xt[:, k * C + c0:k * C + c0 + cn],
                                start=(k == 0),
                                stop=(k == KD - 1),
                            )
                        nc.scalar.activation(
                            ht[:, m * C + c0:m * C + c0 + cn],
                            ps[:, :cn],
                            mybir.ActivationFunctionType.Gelu,
                            bias=b1t[:, m:m + 1],
                        )
                        c0 += cn

            # ---- Phase B: yT = W2^T hT ----
            for d in range(KD):
                w2d = w2p.tile([P, MH * P], f32r, tag="w2d")
                nc.scalar.dma_start(out=w2d[:], in_=w2_d[d])
                c0 = 0
                for cn in chunks:
                    ps = psum.tile([P, 512], mybir.dt.float32, tag="ps")
                    for k in range(MH):
                        nc.tensor.matmul(
                            ps[:, :cn],
                            w2d[:, k * P:(k + 1) * P],
                            ht[:, k * C + c0:k * C + c0 + cn],
                            start=(k == 0),
                            stop=(k == MH - 1),
                        )
                    ot = outp.tile([P, 512], f32, tag="ot")
                    nc.vector.tensor_copy(ot[:, :cn], ps[:, :cn])
                    nc.gpsimd.dma_start(
                        out=yt_d[d, :, c0:c0 + cn], in_=ot[:, :cn]
                    )
                    c0 += cn

    nc.compile()
    return nc


def _get_program(C):
    if C not in _compiled_cache:
        _compiled_cache[C] = _build_program(C)
    return _compiled_cache[C]


def kernel(x, Wg, bg, W1, b1, W2, b2):
    global last_exec_time_ns, last_results
    from concourse.bass_utils import run_bass_kernel_spmd

    x = np.asarray(x, dtype=np.float32)
    Wg = np.asarray(Wg, dtype=np.float32)
    bg = np.asarray(bg, dtype=np.float32)
    W1 = np.asarray(W1, dtype=np.float32)
    b1 = np.asarray(b1, dtype=np.float32)
    W2 = np.asarray(W2, dtype=np.float32)
    b2 = np.asarray(b2, dtype=np.float32)

    B, S, D = x.shape
    T = B * S
    xf = x.reshape(T, D)

    # ---- Router (replicated gate, computed host-side as the dispatch step)
    logits = xf @ Wg + bg
    eidx = np.argmax(logits, axis=-1)

    tok = [np.nonzero(eidx == e)[0] for e in range(N_EXPERTS)]
    counts = [len(t) for t in tok]
    C = max(max(counts), 512)
    C = ((C + 7) // 8) * 8  # mild alignment for DMA friendliness

    nc = _get_program(C)

    # ---- Build per-core pre-tiled inputs
    in_maps = []
    for e in range(N_EXPERTS):
        n_e = counts[e]
        xe = xf[tok[e]]  # [n_e, D]
        xt = np.zeros((P, KD * C), dtype=np.float32)
        xeT = np.ascontiguousarray(xe.T).reshape(KD, P, n_e)
        for k in range(KD):
            xt[:, k * C:k * C + n_e] = xeT[k]
        # lhsT tiles, contiguous per DMA group
        w1 = np.ascontiguousarray(
            W1[e].reshape(KD, P, MH // MG, MG, P)
            .transpose(2, 1, 3, 0, 4)
            .reshape(MH // MG, P, MG * KD * P)
        )
        w2 = np.ascontiguousarray(
            W2[e].reshape(MH, P, KD, P).transpose(2, 1, 0, 3).reshape(KD, P, MH * P)
        )
        b1t = np.ascontiguousarray(b1[e].reshape(MH, P).T)
        in_maps.append({"xt": xt, "w1": w1, "w2": w2, "b1t": b1t})

    trace = os.environ.get("BASS_KERNEL_TRACE", "") == "1"
    if trace:
        try:
            import axon_profile_shim

            axon_profile_shim.install()
        except ImportError:
            pass

    res = run_bass_kernel_spmd(nc, in_maps, list(range(N_CORES)), trace=trace)
    last_exec_time_ns = res.exec_time_ns
    last_results = res

    # ---- Combine: scatter tokens back, add b2 host-side
    out = np.zeros((T, D), dtype=np.float32)
    for e in range(N_EXPERTS):
        n_e = counts[e]
        if n_e == 0:
            continue
        yt = res.results[e]["yt"]  # [KD, P, C]
        ye = yt.reshape(D, C)[:, :n_e].T  # [n_e, D]
        out[tok[e]] = ye + b2[e][None, :]
    return out.reshape(B, S, D)


# revision 11
# speedup vs baseline: 1.0002x; 1.0002x over previous
"""MoE FFN (top-1 routing) Trainium2 kernel — expert-parallel across 8 cores.

Strategy (per the expert-parallel sharding hint): the router gate and the
token dispatch ARE the sharding step, performed on the host inside kernel():
  - host computes router logits (x @ Wg + bg) and argmax expert ids
  - tokens are gathered per expert, padded to capacity C = max expert load
  - core e receives expert e's W1/W2/b1 plus its routed tokens, pre-tiled
    into contiguous-DMA layouts
  - the device runs the full FFN (both matmuls + exact gelu) in float32r
  - host scatters per-expert outputs back (adds b2 there, it is per-token
    constant) and un-shards to the full [B, S, D] output

Device kernel per core (C tokens, D=1024, H=4096):
  phase A: hT[m*128+j, c] = gelu(sum_k W1tile[k,m].T x^T[k] + b1), m in 0..31
  phase B: yT[d*128+j, c] = sum_k W2tile[k,d].T hT[k],             d in 0..7
Both matmuls consume the weights in their natural [K, M] orientation as the
stationary operand, so no transposes are needed anywhere on the device.

DMA ring assignment: w1 stream on the sync HWDGE ring; xt + w2 stream on the
scalar HWDGE ring; output writes on gpsimd SWDGE.  A short burst of scratch
matmuls at kernel start keeps the PE HAM clock-gate warm through the input
load prologue.
"""

import os
import sys

import numpy as np

for _p in ("/opt/trn_rl_repo", "/root/.axon_site/_ro/trn_rl_repo"):
    if os.path.isdir(_p) and _p not in sys.path:
        sys.path.insert(0, _p)

D_MODEL = 1024
D_HIDDEN = 4096
N_EXPERTS = 8
N_CORES = 8
P = 128
KD = D_MODEL // P  # 8 k-chunks over d_model
MH = D_HIDDEN // P  # 32 m-chunks over d_hidden
MG = 2  # m-chunks per w1 DMA (1 MiB transfers)
N_WARM = 5  # scratch f32 matmuls (~1.7us each cold) to warm the PE clock gate

_compiled_cache = {}

# Set by the most recent kernel() call when BASS_KERNEL_TRACE=1: HW exec ns.
last_exec_time_ns = None
last_results = None


def _chunk_sizes(C):
    """Split C token columns into chunks <= 512, as evenly as possible.

    C >= 512 always (max expert load >= 4096/8), so chunks land in
    [256, 512] and float32r matmuls run at full 1 cycle/row speed.
    """
    nch = -(-C // 512)
    base, rem = divmod(C, nch)
    return [base + 1] * rem + [base] * (nch - rem)


def _build_program(C):
    import concourse.mybir as mybir
    import concourse.tile as tile
    from concourse import bacc

    f32 = mybir.dt.float32
    f32r = mybir.dt.float32r

    nc = bacc.Bacc("TRN2", target_bir_lowering=False, debug=False,
                   num_devices=N_CORES)

    # Host-pretiled inputs (layouts chosen so each DMA is contiguous):
    #   xt  [128, KD*C]            xt[p, k*C + c] = x[c, k*128+p]
    #   w1  [MH/MG, 128, MG*KD*128] w1[g, p, (i*KD+k)*128+j] = W1[k*128+p, (g*MG+i)*128+j]
    #   w2  [KD, 128, MH*128]       w2[d, p, k*128+j] = W2[k*128+p, d*128+j]
    #   b1t [128, MH]               b1t[p, m] = b1[m*128+p]
    # Output:
    #   yt  [KD, 128, C]            yt[d, p, c] = y[c, d*128+p]  (pre-b2)
    xt_d = nc.declare_dram_parameter("xt", [P, KD * C], f32r, isOutput=False)
    w1_d = nc.declare_dram_parameter(
        "w1", [MH // MG, P, MG * KD * P], f32r, isOutput=False)
    w2_d = nc.declare_dram_parameter("w2", [KD, P, MH * P], f32r, isOutput=False)
    b1_d = nc.declare_dram_parameter("b1t", [P, MH], f32, isOutput=False)
    yt_d = nc.declare_dram_parameter("yt", [KD, P, C], f32, isOutput=True)

    chunks = _chunk_sizes(C)

    with tile.TileContext(nc) as tc:
        with (
            tc.tile_pool(name="persist", bufs=1) as persist,
            tc.tile_pool(name="w1p", bufs=3) as w1p,
            tc.tile_pool(name="w2p", bufs=3) as w2p,
            tc.tile_pool(name="outp", bufs=4) as outp,
            tc.tile_pool(name="psum", bufs=6, space="PSUM") as psum,
        ):
            # --- PE warm-up: keep HAM at 8/8 through the input-load prologue
            scratch = persist.tile([P, 512], f32)
            nc.vector.memset(scratch[:], 0.0)
            warm_ps = psum.tile([P, 512], mybir.dt.float32, tag="warm", bufs=1)
            for _ in range(N_WARM):
                nc.tensor.matmul(warm_ps[:], scratch[:, :P], scratch[:],
                                 start=True, stop=True)

            # --- input loads
            xt = persist.tile([P, KD * C], f32r)
            nc.scalar.dma_start(out=xt[:], in_=xt_d[:])
            b1t = persist.tile([P, MH], f32)
            nc.gpsimd.dma_start(out=b1t[:], in_=b1_d[:])
            ht = persist.tile([P, MH * C], f32r)

            # w2 tiles are prefetched on the scalar ring: the first bufs-many
            # late in phase A (so they don't contend with the w1 stream),
            # the rest at prefetch distance 3 inside phase B.
            w2_tiles = [None] * KD

            def load_w2(d):
                t = w2p.tile([P, MH * P], f32r, tag="w2d")
                nc.scalar.dma_start(out=t[:], in_=w2_d[d])
                w2_tiles[d] = t

            # ---- Phase A: hT = gelu(W1^T x^T + b1) ----
            for g in range(MH // MG):
                w1g = w1p.tile([P, MG * KD * P], f32r, tag="w1g")
                nc.sync.dma_start(out=w1g[:], in_=w1_d[g])
                if g == 6:
                    load_w2(0)
                elif g == 10:
                    load_w2(1)
                elif g == 13:
                    load_w2(2)
                for i in range(MG):
                    m = g * MG + i
                    c0 = 0
                    for cn in chunks:
                        ps = psum.tile([P, 512], mybir.dt.float32, tag="ps")
                        for k in range(KD):
                            nc.tensor.matmul(
                                ps[:, :cn],
                                w1g[:, (i * KD + k) * P:(i * KD + k + 1) * P],
                                xt[:, k * C + c0:k * C + c0 + cn],
                                start=(k == 0),
                                stop=(k == KD - 1),
                            )
                        nc.scalar.activation(
                            ht[:, m * C + c0:m * C + c0 + cn],
                            ps[:, :cn],
                            mybir.ActivationFunctionType.Gelu,
                            bias=b1t[:, m:m + 1],
                        )
                        c0 += cn

            # ---- Phase B: yT = W2^T hT ----
            for d in range(KD):
                w2d = w2_tiles[d]
                c0 = 0
                for cn in chunks:
                    ps = psum.tile([P, 512], mybir.dt.float32, tag="ps")
                    for k in range(MH):
                        nc.tensor.matmul(
                            ps[:, :cn],
                            w2d[:, k * P:(k + 1) * P],
                            ht[:, k * C + c0:k * C + c0 + cn],
                            start=(k == 0),
                            stop=(k == MH - 1),
                        )
                    ot = outp.tile([P, 512], f32, tag="ot")
                    nc.vector.tensor_copy(ot[:, :cn], ps[:, :cn])
                    nc.gpsimd.dma_start(
                        out=yt_d[d, :, c0:c0 + cn], in_=ot[:, :cn]
                    )
                    c0 += cn
                if d + 3 < KD:
                    load_w2(d + 3)

    nc.compile()
    return nc


def _get_program(C):
    if C not in _compiled_cache:
        _compiled_cache[C] = _build_program(C)
    return _compiled_cache[C]


def kernel(x, Wg, bg, W1, b1, W2, b2):
    global last_exec_time_ns, last_results
    from concourse.bass_utils import run_bass_kernel_spmd

    x = np.asarray(x, dtype=np.float32)
    Wg = np.asarray(Wg, dtype=np.float32)
    bg = np.asarray(bg, dtype=np.float32)
    W1 = np.asarray(W1, dtype=np.float32)
    b1 = np.asarray(b1, dtype=np.float32)
    W2 = np.asarray(W2, dtype=np.float32)
    b2 = np.asarray(b2, dtype=np.float32)

    B, S, D = x.shape
    T = B * S
    xf = x.reshape(T, D)

    # ---- Router (replicated gate, computed host-side as the dispatch step)
    logits = xf @ Wg + bg
    eidx = np.argmax(logits, axis=-1)

    tok = [np.nonzero(eidx == e)[0] for e in range(N_EXPERTS)]
    counts = [len(t) for t in tok]
    C = max(max(counts), 512)
    C = ((C + 7) // 8) * 8  # mild alignment for DMA friendliness

    nc = _get_program(C)

    # ---- Build per-core pre-tiled inputs
    in_maps = []
    for e in range(N_EXPERTS):
        n_e = counts[e]
        xe = xf[tok[e]]  # [n_e, D]
        xt = np.zeros((P, KD * C), dtype=np.float32)
        xeT = np.ascontiguousarray(xe.T).reshape(KD, P, n_e)
        for k in range(KD):
            xt[:, k * C:k * C + n_e] = xeT[k]
        # lhsT tiles, contiguous per DMA group
        w1 = np.ascontiguousarray(
            W1[e].reshape(KD, P, MH // MG, MG, P)
            .transpose(2, 1, 3, 0, 4)
            .reshape(MH // MG, P, MG * KD * P)
        )
        w2 = np.ascontiguousarray(
            W2[e].reshape(MH, P, KD, P).transpose(2, 1, 0, 3).reshape(KD, P, MH * P)
        )
        b1t = np.ascontiguousarray(b1[e].reshape(MH, P).T)
        in_maps.append({"xt": xt, "w1": w1, "w2": w2, "b1t": b1t})

    trace = os.environ.get("BASS_KERNEL_TRACE", "") == "1"
    if trace:
        try:
            import axon_profile_shim

            axon_profile_shim.install()
        except ImportError:
            pass

    res = run_bass_kernel_spmd(nc, in_maps, list(range(N_CORES)), trace=trace)
    last_exec_time_ns = res.exec_time_ns
    last_results = res

    # ---- Combine: scatter tokens back, add b2 host-side
    out = np.zeros((T, D), dtype=np.float32)
    for e in range(N_EXPERTS):
        n_e = counts[e]
        if n_e == 0:
            continue
        yt = res.results[e]["yt"]  # [KD, P, C]
        ye = yt.reshape(D, C)[:, :n_e].T  # [n_e, D]
        out[tok[e]] = ye + b2[e][None, :]
    return out.reshape(B, S, D)


# revision 13
# speedup vs baseline: 1.0255x; 1.0253x over previous
"""MoE FFN (top-1 routing) Trainium2 kernel — expert-parallel across 8 cores.

Strategy (per the expert-parallel sharding hint): the router gate and the
token dispatch ARE the sharding step, performed on the host inside kernel():
  - host computes router logits (x @ Wg + bg) and argmax expert ids
  - tokens are gathered per expert, padded to capacity C = max expert load
  - core e receives expert e's W1/W2/b1 plus its routed tokens, pre-tiled
    into contiguous-DMA layouts
  - the device runs the full FFN (both matmuls + exact gelu) in float32r
  - host scatters per-expert outputs back (adds b2 there, it is per-token
    constant) and un-shards to the full [B, S, D] output

Device kernel per core (C tokens, D=1024, H=4096):
  phase A: hT[m*128+j, c] = gelu(sum_k W1tile[k,m].T x^T[k] + b1), m in 0..31
  phase B: yT[d*128+j, c] = sum_k W2tile[k,d].T hT[k],             d in 0..7
Both matmuls consume the weights in their natural [K, M] orientation as the
stationary operand, so no transposes are needed anywhere on the device.

DMA ring assignment: w1 stream on the sync HWDGE ring; xt + w2 stream on the
scalar HWDGE ring; output writes on gpsimd SWDGE.  A short burst of scratch
matmuls at kernel start keeps the PE HAM clock-gate warm through the input
load prologue.
"""

import os
import sys

import numpy as np

for _p in ("/opt/trn_rl_repo", "/root/.axon_site/_ro/trn_rl_repo"):
    if os.path.isdir(_p) and _p not in sys.path:
        sys.path.insert(0, _p)

D_MODEL = 1024
D_HIDDEN = 4096
N_EXPERTS = 8
N_CORES = 8
P = 128
KD = D_MODEL // P  # 8 k-chunks over d_model
MH = D_HIDDEN // P  # 32 m-chunks over d_hidden
MG = 2  # m-chunks per w1 DMA (1 MiB transfers)
N_WARM = 5  # scratch f32 matmuls (~1.7us each cold) to warm the PE clock gate

_compiled_cache = {}

# Set by the most recent kernel() call when BASS_KERNEL_TRACE=1: HW exec ns.
last_exec_time_ns = None
last_results = None


def _chunk_sizes(C):
    """Split C token columns into chunks <= 512, as evenly as possible.

    C >= 512 always (max expert load >= 4096/8), so chunks land in
    [256, 512] and float32r matmuls run at full 1 cycle/row speed.
    """
    nch = -(-C // 512)
    base, rem = divmod(C, nch)
    return [base + 1] * rem + [base] * (nch - rem)


def _build_program(C):
    import concourse.mybir as mybir
    import concourse.tile as tile
    from concourse import bacc

    f32 = mybir.dt.float32
    f32r = mybir.dt.float32r

    nc = bacc.Bacc("TRN2", target_bir_lowering=False, debug=False,
                   num_devices=N_CORES)

    # Host-pretiled inputs (layouts chosen so each DMA is contiguous):
    #   xt  [128, KD*C]            xt[p, k*C + c] = x[c, k*128+p]
    #   w1  [MH/MG, 128, MG*KD*128] w1[g, p, (i*KD+k)*128+j] = W1[k*128+p, (g*MG+i)*128+j]
    #   w2  [KD, 128, MH*128]       w2[d, p, k*128+j] = W2[k*128+p, d*128+j]
    #   b1t [128, MH]               b1t[p, m] = b1[m*128+p]
    # Output:
    #   yt  [KD, 128, C]            yt[d, p, c] = y[c, d*128+p]  (pre-b2)
    xt_d = nc.declare_dram_parameter("xt", [P, KD * C], f32r, isOutput=False)
    w1_d = nc.declare_dram_parameter(
        "w1", [MH // MG, P, MG * KD * P], f32r, isOutput=False)
    w2_d = nc.declare_dram_parameter("w2", [KD, P, MH * P], f32r, isOutput=False)
    b1_d = nc.declare_dram_parameter("b1t", [P, MH], f32, isOutput=False)
    yt_d = nc.declare_dram_parameter("yt", [KD, P, C], f32, isOutput=True)

    chunks = _chunk_sizes(C)

    with tile.TileContext(nc) as tc:
        with (
            tc.tile_pool(name="persist", bufs=1) as persist,
            tc.tile_pool(name="w1p", bufs=5) as w1p,
            tc.tile_pool(name="w2p", bufs=3) as w2p,
            tc.tile_pool(name="outp", bufs=4) as outp,
            tc.tile_pool(name="psum", bufs=6, space="PSUM") as psum,
        ):
            # --- PE warm-up: keep HAM at 8/8 through the input-load prologue
            scratch = persist.tile([P, 512], f32)
            nc.vector.memset(scratch[:], 0.0)
            warm_ps = psum.tile([P, 512], mybir.dt.float32, tag="warm", bufs=1)
            for _ in range(N_WARM):
                nc.tensor.matmul(warm_ps[:], scratch[:, :P], scratch[:],
                                 start=True, stop=True)

            # --- input loads: xt split across the sync HWDGE ring (ahead of
            # the w1 stream) and the gpsimd SWDGE ring so both halves land
            # while the PE warm-up is still running.
            xt = persist.tile([P, KD * C], f32r)
            half = ((KD * C) // 2) // P * P
            nc.sync.dma_start(out=xt[:, :half], in_=xt_d[:, :half])
            b1t = persist.tile([P, MH], f32)
            nc.gpsimd.dma_start(out=b1t[:], in_=b1_d[:])
            nc.gpsimd.dma_start(out=xt[:, half:], in_=xt_d[:, half:])
            ht = persist.tile([P, MH * C], f32r)

            # w2 tiles are prefetched on the scalar ring: the first bufs-many
            # late in phase A (so they don't contend with the w1 stream),
            # the rest at prefetch distance 3 inside phase B.
            w2_tiles = [None] * KD

            def load_w2(d):
                t = w2p.tile([P, MH * P], f32r, tag="w2d")
                nc.scalar.dma_start(out=t[:], in_=w2_d[d])
                w2_tiles[d] = t

            # ---- Phase A: hT = gelu(W1^T x^T + b1) ----
            for g in range(MH // MG):
                w1g = w1p.tile([P, MG * KD * P], f32r, tag="w1g")
                nc.sync.dma_start(out=w1g[:], in_=w1_d[g])
                if g == 6:
                    load_w2(0)
                elif g == 10:
                    load_w2(1)
                elif g == 13:
                    load_w2(2)
                for i in range(MG):
                    m = g * MG + i
                    c0 = 0
                    for cn in chunks:
                        ps = psum.tile([P, 512], mybir.dt.float32, tag="ps")
                        for k in range(KD):
                            nc.tensor.matmul(
                                ps[:, :cn],
                                w1g[:, (i * KD + k) * P:(i * KD + k + 1) * P],
                                xt[:, k * C + c0:k * C + c0 + cn],
                                start=(k == 0),
                                stop=(k == KD - 1),
                            )
                        nc.scalar.activation(
                            ht[:, m * C + c0:m * C + c0 + cn],
                            ps[:, :cn],
                            mybir.ActivationFunctionType.Gelu,
                            bias=b1t[:, m:m + 1],
                        )
                        c0 += cn

            # ---- Phase B: yT = W2^T hT ----
            for d in range(KD):
                w2d = w2_tiles[d]
                c0 = 0
                for cn in chunks:
                    ps = psum.tile([P, 512], mybir.dt.float32, tag="ps")
                    for k in range(MH):
                        nc.tensor.matmul(
                            ps[:, :cn],
                            w2d[:, k * P:(k + 1) * P],
                            ht[:, k * C + c0:k * C + c0 + cn],
                            start=(k == 0),
                            stop=(k == MH - 1),
                        )
                    ot = outp.tile([P, 512], f32, tag="ot")
                    nc.vector.tensor_copy(ot[:, :cn], ps[:, :cn])
                    nc.gpsimd.dma_start(
                        out=yt_d[d, :, c0:c0 + cn], in_=ot[:, :cn]
                    )
                    c0 += cn
                if d + 3 < KD:
                    load_w2(d + 3)

    nc.compile()
    return nc


def _get_program(C):
    if C not in _compiled_cache:
        _compiled_cache[C] = _build_program(C)
    return _compiled_cache[C]


def kernel(x, Wg, bg, W1, b1, W2, b2):
    global last_exec_time_ns, last_results
    from concourse.bass_utils import run_bass_kernel_spmd

    x = np.asarray(x, dtype=np.float32)
    Wg = np.asarray(Wg, dtype=np.float32)
    bg = np.asarray(bg, dtype=np.float32)
    W1 = np.asarray(W1, dtype=np.float32)
    b1 = np.asarray(b1, dtype=np.float32)
    W2 = np.asarray(W2, dtype=np.float32)
    b2 = np.asarray(b2, dtype=np.float32)

    B, S, D = x.shape
    T = B * S
    xf = x.reshape(T, D)

    # ---- Router (replicated gate, computed host-side as the dispatch step)
    logits = xf @ Wg + bg
    eidx = np.argmax(logits, axis=-1)

    tok = [np.nonzero(eidx == e)[0] for e in range(N_EXPERTS)]
    counts = [len(t) for t in tok]
    C = max(max(counts), 512)
    C = ((C + 7) // 8) * 8  # mild alignment for DMA friendliness

    nc = _get_program(C)

    # ---- Build per-core pre-tiled inputs
    in_maps = []
    for e in range(N_EXPERTS):
        n_e = counts[e]
        xe = xf[tok[e]]  # [n_e, D]
        xt = np.zeros((P, KD * C), dtype=np.float32)
        xeT = np.ascontiguousarray(xe.T).reshape(KD, P, n_e)
        for k in range(KD):
            xt[:, k * C:k * C + n_e] = xeT[k]
        # lhsT tiles, contiguous per DMA group
        w1 = np.ascontiguousarray(
            W1[e].reshape(KD, P, MH // MG, MG, P)
            .transpose(2, 1, 3, 0, 4)
            .reshape(MH // MG, P, MG * KD * P)
        )
        w2 = np.ascontiguousarray(
            W2[e].reshape(MH, P, KD, P).transpose(2, 1, 0, 3).reshape(KD, P, MH * P)
        )
        b1t = np.ascontiguousarray(b1[e].reshape(MH, P).T)
        in_maps.append({"xt": xt, "w1": w1, "w2": w2, "b1t": b1t})

    trace = os.environ.get("BASS_KERNEL_TRACE", "") == "1"
    if trace:
        try:
            import axon_profile_shim

            axon_profile_shim.install()
        except ImportError:
            pass

    res = run_bass_kernel_spmd(nc, in_maps, list(range(N_CORES)), trace=trace)
    last_exec_time_ns = res.exec_time_ns
    last_results = res

    # ---- Combine: scatter tokens back, add b2 host-side
    out = np.zeros((T, D), dtype=np.float32)
    for e in range(N_EXPERTS):
        n_e = counts[e]
        if n_e == 0:
            continue
        yt = res.results[e]["yt"]  # [KD, P, C]
        ye = yt.reshape(D, C)[:, :n_e].T  # [n_e, D]
        out[tok[e]] = ye + b2[e][None, :]
    return out.reshape(B, S, D)


# revision 14
# speedup vs baseline: 1.0416x; 1.0158x over previous
"""MoE FFN (top-1 routing) Trainium2 kernel — expert-parallel across 8 cores.

Strategy (per the expert-parallel sharding hint): the router gate and the
token dispatch ARE the sharding step, performed on the host inside kernel():
  - host computes router logits (x @ Wg + bg) and argmax expert ids
  - tokens are gathered per expert, padded to capacity C = max expert load
  - core e receives expert e's W1/W2/b1 plus its routed tokens, pre-tiled
    into contiguous-DMA layouts
  - the device runs the full FFN (both matmuls + exact gelu) in float32r
  - host scatters per-expert outputs back (adds b2 there, it is per-token
    constant) and un-shards to the full [B, S, D] output

Device kernel per core (C tokens, D=1024, H=4096):
  phase A: hT[m*128+j, c] = gelu(sum_k W1tile[k,m].T x^T[k] + b1), m in 0..31
  phase B: yT[d*128+j, c] = sum_k W2tile[k,d].T hT[k],             d in 0..7
Both matmuls consume the weights in their natural [K, M] orientation as the
stationary operand, so no transposes are needed anywhere on the device.

DMA ring assignment: w1 stream on the sync HWDGE ring; xt + w2 stream on the
scalar HWDGE ring; output writes on gpsimd SWDGE.  A short burst of scratch
matmuls at kernel start keeps the PE HAM clock-gate warm through the input
load prologue.
"""

import os
import sys

import numpy as np

for _p in ("/opt/trn_rl_repo", "/root/.axon_site/_ro/trn_rl_repo"):
    if os.path.isdir(_p) and _p not in sys.path:
        sys.path.insert(0, _p)

D_MODEL = 1024
D_HIDDEN = 4096
N_EXPERTS = 8
N_CORES = 8
P = 128
KD = D_MODEL // P  # 8 k-chunks over d_model
MH = D_HIDDEN // P  # 32 m-chunks over d_hidden
MG = 2  # m-chunks per w1 DMA (1 MiB transfers)
N_WARM = 5  # scratch f32 matmuls (~1.7us each cold) to warm the PE clock gate

_compiled_cache = {}

# Set by the most recent kernel() call when BASS_KERNEL_TRACE=1: HW exec ns.
last_exec_time_ns = None
last_results = None


def _chunk_sizes(C):
    """Split C token columns into chunks <= 512, as evenly as possible.

    C >= 512 always (max expert load >= 4096/8), so chunks land in
    [256, 512] and float32r matmuls run at full 1 cycle/row speed.
    """
    nch = -(-C // 512)
    base, rem = divmod(C, nch)
    return [base + 1] * rem + [base] * (nch - rem)


def _build_program(C):
    import concourse.mybir as mybir
    import concourse.tile as tile
    from concourse import bacc

    f32 = mybir.dt.float32
    f32r = mybir.dt.float32r

    nc = bacc.Bacc("TRN2", target_bir_lowering=False, debug=False,
                   num_devices=N_CORES)

    # Host-pretiled inputs (layouts chosen so each DMA is contiguous):
    #   xt  [128, KD*C]            xt[p, k*C + c] = x[c, k*128+p]
    #   w1  [MH/MG, 128, MG*KD*128] w1[g, p, (i*KD+k)*128+j] = W1[k*128+p, (g*MG+i)*128+j]
    #   w2  [KD, 128, MH*128]       w2[d, p, k*128+j] = W2[k*128+p, d*128+j]
    #   b1t [128, MH]               b1t[p, m] = b1[m*128+p]
    # Output:
    #   yt  [KD, 128, C]            yt[d, p, c] = y[c, d*128+p]  (pre-b2)
    xt_d = nc.declare_dram_parameter("xt", [P, KD * C], f32r, isOutput=False)
    w1_d = nc.declare_dram_parameter(
        "w1", [MH // MG, P, MG * KD * P], f32r, isOutput=False)
    w2_d = nc.declare_dram_parameter("w2", [KD, P, MH * P], f32r, isOutput=False)
    b1_d = nc.declare_dram_parameter("b1t", [P, MH], f32, isOutput=False)
    yt_d = nc.declare_dram_parameter("yt", [KD, P, C], f32, isOutput=True)

    chunks = _chunk_sizes(C)

    with tile.TileContext(nc) as tc:
        with (
            tc.tile_pool(name="persist", bufs=1) as persist,
            tc.tile_pool(name="w1p", bufs=5) as w1p,
            tc.tile_pool(name="w2p", bufs=3) as w2p,
            tc.tile_pool(name="outp", bufs=4) as outp,
            tc.tile_pool(name="psum", bufs=6, space="PSUM") as psum,
        ):
            # --- PE warm-up: keep HAM at 8/8 through the input-load prologue
            scratch = persist.tile([P, 512], f32)
            nc.vector.memset(scratch[:], 0.0)
            warm_ps = psum.tile([P, 512], mybir.dt.float32, tag="warm", bufs=1)
            for _ in range(N_WARM):
                nc.tensor.matmul(warm_ps[:], scratch[:, :P], scratch[:],
                                 start=True, stop=True)

            # --- input loads: xt split across the sync HWDGE ring (ahead of
            # the w1 stream) and the gpsimd SWDGE ring so both halves land
            # while the PE warm-up is still running.
            xt = persist.tile([P, KD * C], f32r)
            half = ((KD * C) // 2) // P * P
            nc.sync.dma_start(out=xt[:, :half], in_=xt_d[:, :half])
            b1t = persist.tile([P, MH], f32)
            nc.gpsimd.dma_start(out=b1t[:], in_=b1_d[:])
            nc.scalar.dma_start(out=xt[:, half:], in_=xt_d[:, half:])
            ht = persist.tile([P, MH * C], f32r)

            # w2 tiles are prefetched on the scalar ring: the first bufs-many
            # late in phase A (so they don't contend with the w1 stream),
            # the rest at prefetch distance 3 inside phase B.
            w2_tiles = [None] * KD

            def load_w2(d):
                t = w2p.tile([P, MH * P], f32r, tag="w2d")
                nc.scalar.dma_start(out=t[:], in_=w2_d[d])
                w2_tiles[d] = t

            # ---- Phase A: hT = gelu(W1^T x^T + b1) ----
            for g in range(MH // MG):
                w1g = w1p.tile([P, MG * KD * P], f32r, tag="w1g")
                nc.sync.dma_start(out=w1g[:], in_=w1_d[g])
                if g == 6:
                    load_w2(0)
                elif g == 10:
                    load_w2(1)
                elif g == 13:
                    load_w2(2)
                for i in range(MG):
                    m = g * MG + i
                    c0 = 0
                    for cn in chunks:
                        ps = psum.tile([P, 512], mybir.dt.float32, tag="ps")
                        for k in range(KD):
                            nc.tensor.matmul(
                                ps[:, :cn],
                                w1g[:, (i * KD + k) * P:(i * KD + k + 1) * P],
                                xt[:, k * C + c0:k * C + c0 + cn],
                                start=(k == 0),
                                stop=(k == KD - 1),
                            )
                        nc.scalar.activation(
                            ht[:, m * C + c0:m * C + c0 + cn],
                            ps[:, :cn],
                            mybir.ActivationFunctionType.Gelu,
                            bias=b1t[:, m:m + 1],
                        )
                        c0 += cn

            # ---- Phase B: yT = W2^T hT ----
            for d in range(KD):
                w2d = w2_tiles[d]
                c0 = 0
                for cn in chunks:
                    ps = psum.tile([P, 512], mybir.dt.float32, tag="ps")
                    for k in range(MH):
                        nc.tensor.matmul(
                            ps[:, :cn],
                            w2d[:, k * P:(k + 1) * P],
                            ht[:, k * C + c0:k * C + c0 + cn],
                            start=(k == 0),
                            stop=(k == MH - 1),
                        )
                    ot = outp.tile([P, 512], f32, tag="ot")
                    nc.vector.tensor_copy(ot[:, :cn], ps[:, :cn])
                    nc.gpsimd.dma_start(
                        out=yt_d[d, :, c0:c0 + cn], in_=ot[:, :cn]
                    )
                    c0 += cn
                if d + 3 < KD:
                    load_w2(d + 3)

    nc.compile()
    return nc


def _get_program(C):
    if C not in _compiled_cache:
        _compiled_cache[C] = _build_program(C)
    return _compiled_cache[C]


def kernel(x, Wg, bg, W1, b1, W2, b2):
    global last_exec_time_ns, last_results
    from concourse.bass_utils import run_bass_kernel_spmd

    x = np.asarray(x, dtype=np.float32)
    Wg = np.asarray(Wg, dtype=np.float32)
    bg = np.asarray(bg, dtype=np.float32)
    W1 = np.asarray(W1, dtype=np.float32)
    b1 = np.asarray(b1, dtype=np.float32)
    W2 = np.asarray(W2, dtype=np.float32)
    b2 = np.asarray(b2, dtype=np.float32)

    B, S, D = x.shape
    T = B * S
    xf = x.reshape(T, D)

    # ---- Router (replicated gate, computed host-side as the dispatch step)
    logits = xf @ Wg + bg
    eidx = np.argmax(logits, axis=-1)

    tok = [np.nonzero(eidx == e)[0] for e in range(N_EXPERTS)]
    counts = [len(t) for t in tok]
    C = max(max(counts), 512)
    C = ((C + 7) // 8) * 8  # mild alignment for DMA friendliness

    nc = _get_program(C)

    # ---- Build per-core pre-tiled inputs
    in_maps = []
    for e in range(N_EXPERTS):
        n_e = counts[e]
        xe = xf[tok[e]]  # [n_e, D]
        xt = np.zeros((P, KD * C), dtype=np.float32)
        xeT = np.ascontiguousarray(xe.T).reshape(KD, P, n_e)
        for k in range(KD):
            xt[:, k * C:k * C + n_e] = xeT[k]
        # lhsT tiles, contiguous per DMA group
        w1 = np.ascontiguousarray(
            W1[e].reshape(KD, P, MH // MG, MG, P)
            .transpose(2, 1, 3, 0, 4)
            .reshape(MH // MG, P, MG * KD * P)
        )
        w2 = np.ascontiguousarray(
            W2[e].reshape(MH, P, KD, P).transpose(2, 1, 0, 3).reshape(KD, P, MH * P)
        )
        b1t = np.ascontiguousarray(b1[e].reshape(MH, P).T)
        in_maps.append({"xt": xt, "w1": w1, "w2": w2, "b1t": b1t})

    trace = os.environ.get("BASS_KERNEL_TRACE", "") == "1"
    if trace:
        try:
            import axon_profile_shim

            axon_profile_shim.install()
        except ImportError:
            pass

    res = run_bass_kernel_spmd(nc, in_maps, list(range(N_CORES)), trace=trace)
    last_exec_time_ns = res.exec_time_ns
    last_results = res

    # ---- Combine: scatter tokens back, add b2 host-side
    out = np.zeros((T, D), dtype=np.float32)
    for e in range(N_EXPERTS):
        n_e = counts[e]
        if n_e == 0:
            continue
        yt = res.results[e]["yt"]  # [KD, P, C]
        ye = yt.reshape(D, C)[:, :n_e].T  # [n_e, D]
        out[tok[e]] = ye + b2[e][None, :]
    return out.reshape(B, S, D)
